# revision 7
# baseline (speedup 1.0000x reference)
"""GPTQ-style 4-bit dequantizer on 8 TRN2 NeuronCores.

Strategy (column-parallel, per the N-axis sharding):
  - Shard qweight/scales/qzeros/output along N across 8 cores; g_idx replicated.
  - Per core (shard N_S = 1376 columns):
      * unpack qzeros -> zeros, compute negsz = -(zeros*scales)  [32, N_S], tiny
      * build one-hot(g_idx) [32, 4096] on device (iota + is_equal), laid out in
        (t, j) block order so matmul lhsT slices are contiguous
      * for each packed row-tile t (128 rows of qweight) and shift j (0..7):
          w = (qw_t >> 4j) & 0xF          one fused tensor_scalar (DVE)
          sS = onehot_tj^T @ scales        PE matmul gather -> PSUM
          sZ = onehot_tj^T @ negsz         PE matmul gather -> PSUM
          out_rows(8*kpf+j) = w * sS + sZ  two tensor_tensor ops (DVE)
      * strided-row DMA store (rows 8k+j), 5504 B per row descriptor.
"""

import numpy as np
from contextlib import ExitStack

import concourse.bacc as bacc
import concourse.bass as bass
import concourse.tile as tile
import concourse.mybir as mybir
from concourse.bass_utils import run_bass_kernel_spmd

K = 4096          # input features (rows of dequantized weight)
N = 11008         # output features
G = 32            # quant groups
PF = 8            # int32 packs 8 nibbles
MAXQ = 0xF
NCORES = 8
NS = N // NCORES        # 1376 columns per core
KP = K // PF            # 512 packed rows
NZS = NS // PF          # 172 packed qzero columns per core
KT = KP // 128          # 4 packed row-tiles
CHUNKS = [(0, 512), (512, 512), (1024, NS - 1024)]  # matmul N<=512 chunks

f32 = mybir.dt.float32
i32 = mybir.dt.int32
Alu = mybir.AluOpType

_module_cache = {}


def build_module(n_ktiles=KT):
    nc = bacc.Bacc("TRN2", target_bir_lowering=False, debug=False,
                   num_devices=NCORES)
    qw_d = nc.dram_tensor("qweight", [KP, NS], i32, kind="ExternalInput")
    sc_d = nc.dram_tensor("scales", [G, NS], f32, kind="ExternalInput")
    qz_d = nc.dram_tensor("qzeros", [G, NZS], i32, kind="ExternalInput")
    gi_d = nc.dram_tensor("g_idx", [1, K], i32, kind="ExternalInput")
    out_d = nc.dram_tensor("out", [K, NS], f32, kind="ExternalOutput")

    with tile.TileContext(nc) as tc, ExitStack() as ctx:
        const = ctx.enter_context(tc.tile_pool(name="const", bufs=1))
        qwp = ctx.enter_context(tc.tile_pool(name="qw", bufs=2))
        wfp = ctx.enter_context(tc.tile_pool(name="wf", bufs=2))
        prodp = ctx.enter_context(tc.tile_pool(name="prod", bufs=3))
        outp = ctx.enter_context(tc.tile_pool(name="out", bufs=2))
        psS_p = ctx.enter_context(tc.tile_pool(name="psS", bufs=3, space="PSUM"))
        psZ_p = ctx.enter_context(tc.tile_pool(name="psZ", bufs=3, space="PSUM"))

        # ---- constants / preprocompute ----
        scales_sb = const.tile([G, NS], f32)
        nc.sync.dma_start(scales_sb[:], sc_d.ap())
        qz_sb = const.tile([G, NZS], i32)
        nc.sync.dma_start(qz_sb[:], qz_d.ap())

        # g_idx broadcast to 32 partitions
        g_b = const.tile([G, K], i32)
        nc.sync.dma_start(g_b[:], bass.AP(gi_d, 0, [[0, G], [1, K]]))

        iota_col = const.tile([G, 1], f32)
        nc.gpsimd.iota(iota_col[:], [[0, 1]], channel_multiplier=1,
                       allow_small_or_imprecise_dtypes=True)

        # one-hot in (t, j) block order: block u=t*8+j, col p <-> k = 1024t+8p+j
        onehot = const.tile([G, K], f32)
        g4 = g_b[:].rearrange("p (t q e) -> p t q e", t=KT, q=128, e=PF)
        for t in range(KT):
            for j in range(PF):
                u = t * PF + j
                nc.vector.tensor_scalar(
                    onehot[:, u * 128:(u + 1) * 128], g4[:, t, :, j],
                    iota_col[:], None, op0=Alu.is_equal)

        # unpack zeros (int32, strided by 8), then negsz = -(z * s)
        zeros_i = const.tile([G, NS], i32)
        z3 = zeros_i[:].rearrange("p (c e) -> p c e", e=PF)
        for jz in range(PF):
            nc.vector.tensor_scalar(
                z3[:, :, jz], qz_sb[:], 4 * jz, MAXQ,
                op0=Alu.logical_shift_right, op1=Alu.bitwise_and)
        negsz = const.tile([G, NS], f32)
        nc.vector.scalar_tensor_tensor(
            negsz[:], zeros_i[:], -1.0, scales_sb[:],
            op0=Alu.mult, op1=Alu.mult)

        # ---- main loop ----
        for t in range(n_ktiles):
            qw_t = qwp.tile([128, NS], i32)
            nc.sync.dma_start(qw_t[:], qw_d.ap()[t * 128:(t + 1) * 128, :])
            for j in range(PF):
                u = t * PF + j
                wf = wfp.tile([128, NS], i32)
                nc.vector.tensor_scalar(
                    wf[:], qw_t[:], 4 * j, MAXQ,
                    op0=Alu.logical_shift_right, op1=Alu.bitwise_and)
                ot = outp.tile([128, NS], f32)
                lhs = onehot[:, u * 128:(u + 1) * 128]
                for (c0, cw) in CHUNKS:
                    psS = psS_p.tile([128, cw], f32, tag="psS")
                    nc.tensor.matmul(psS[:], lhs, scales_sb[:, c0:c0 + cw],
                                     start=True, stop=True)
                    psZ = psZ_p.tile([128, cw], f32, tag="psZ")
                    nc.tensor.matmul(psZ[:], lhs, negsz[:, c0:c0 + cw],
                                     start=True, stop=True)
                    prod = prodp.tile([128, 512], f32, tag="prod")
                    nc.vector.tensor_tensor(
                        prod[:, :cw], wf[:, c0:c0 + cw], psS[:], op=Alu.mult)
                    nc.vector.tensor_tensor(
                        ot[:, c0:c0 + cw], prod[:, :cw], psZ[:], op=Alu.add)
                out4 = out_d.ap().rearrange("(t q e) n -> t q e n",
                                            t=KT, q=128, e=PF)
                nc.sync.dma_start(out4[t, :, j, :], ot[:])

    nc.compile()
    return nc


def get_module():
    if "nc" not in _module_cache:
        _module_cache["nc"] = build_module()
    return _module_cache["nc"]


def kernel(qweight, qzeros, scales, g_idx):
    qweight = np.ascontiguousarray(qweight, dtype=np.int32)
    qzeros = np.ascontiguousarray(qzeros, dtype=np.int32)
    scales = np.ascontiguousarray(scales, dtype=np.float32)
    g_idx_2d = np.ascontiguousarray(g_idx, dtype=np.int32).reshape(1, K)

    nc = get_module()
    in_maps = []
    for c in range(NCORES):
        nlo, nhi = c * NS, (c + 1) * NS
        in_maps.append({
            "qweight": np.ascontiguousarray(qweight[:, nlo:nhi]),
            "scales": np.ascontiguousarray(scales[:, nlo:nhi]),
            "qzeros": np.ascontiguousarray(qzeros[:, c * NZS:(c + 1) * NZS]),
            "g_idx": g_idx_2d,
        })
    res = run_bass_kernel_spmd(nc, in_maps, list(range(NCORES))).results
    out = np.concatenate([res[c]["out"] for c in range(NCORES)], axis=1)
    return np.ascontiguousarray(out, dtype=np.float32)


# revision 15
# speedup vs baseline: 1.8151x; 1.8151x over previous
"""GPTQ-style 4-bit dequantizer on 8 TRN2 NeuronCores.

Strategy (column-parallel per the N-axis sharding hint):
  - Shard qweight/scales/qzeros/output along N across 8 cores; g_idx replicated.
  - Per core (shard N_S = 1376 columns), natural row-major output layout:
      * unpack qzeros -> zeros; negsz = -(zeros*scales)  [32, N_S] (tiny)
      * split scales and negsz into 3 exact bf16 addends (8+8+8 mantissa bits)
      * one-hot(g_idx) [32, 4096] built on device, replicated to 3 partition
        strips so K=32 matmuls row-pack into the 128x128 PE array
      * for each 128-row packed tile t and shift j:
          w   = (qw_t >> 4j) & 0xF                    fused tensor_scalar (DVE)
          sS  = sum of 3 bf16 one-hot matmuls          -> PSUM (exact scales row gather)
          out = w * sS                                 one tensor_tensor (DVE) -> PSUM
          out += sum of 3 bf16 one-hot matmuls of negsz   (PE accumulates onto the
                DVE product; per-slot dummy matmuls set has_written once at start)
          ot  = copy(out)                              ACT PSUM->SBUF
      * strided-row DMA store (rows 8*kpf+j), 5504 B per row descriptor.
"""

import numpy as np
from contextlib import ExitStack

import concourse.bacc as bacc
import concourse.bass as bass
import concourse.tile as tile
import concourse.mybir as mybir
from concourse.bass_utils import run_bass_kernel_spmd

K = 4096          # input features (rows of dequantized weight)
N = 11008         # output features
G = 32            # quant groups
PF = 8            # int32 packs 8 nibbles
MAXQ = 0xF
NCORES = 8
NS = N // NCORES        # 1376 columns per core
KP = K // PF            # 512 packed rows
NZS = NS // PF          # 172 packed qzero columns per core
KT = KP // 128          # 4 packed row-tiles
CHUNKS = [(0, 512), (512, 512), (1024, NS - 1024)]  # matmul N<=512 chunks
NSLOT = 3               # persistent PSUM output slots

f32 = mybir.dt.float32
bf16 = mybir.dt.bfloat16
i32 = mybir.dt.int32
Alu = mybir.AluOpType

_module_cache = {}


def _split3(nc, dst, src, tmp_a, tmp_b):
    """dst[0:32]+dst[32:64]+dst[64:96] (bf16) == src (f32) exactly."""
    nc.vector.tensor_copy(dst[0:32, :], src)            # hi = bf16(src)
    nc.vector.tensor_copy(tmp_a[:], dst[0:32, :])       # f32(hi)
    nc.vector.tensor_sub(tmp_b[:], src, tmp_a[:])       # r1 = src - hi (exact)
    nc.vector.tensor_copy(dst[32:64, :], tmp_b[:])      # mid = bf16(r1)
    nc.vector.tensor_copy(tmp_a[:], dst[32:64, :])      # f32(mid)
    nc.vector.tensor_sub(tmp_a[:], tmp_b[:], tmp_a[:])  # r2 = r1 - mid (exact)
    nc.vector.tensor_copy(dst[64:96, :], tmp_a[:])      # lo = bf16(r2) == r2


def build_module(n_ktiles=KT):
    nc = bacc.Bacc("TRN2", target_bir_lowering=False, debug=False,
                   num_devices=NCORES)
    qw_d = nc.dram_tensor("qweight", [KP, NS], i32, kind="ExternalInput")
    sc_d = nc.dram_tensor("scales", [G, NS], f32, kind="ExternalInput")
    qz_d = nc.dram_tensor("qzeros", [G, NZS], i32, kind="ExternalInput")
    gi_d = nc.dram_tensor("g_idx", [1, K], i32, kind="ExternalInput")
    out_d = nc.dram_tensor("out", [K, NS], f32, kind="ExternalOutput")

    with tile.TileContext(nc) as tc, ExitStack() as ctx:
        const = ctx.enter_context(tc.tile_pool(name="const", bufs=1))
        qwp = ctx.enter_context(tc.tile_pool(name="qw", bufs=2))
        wfp = ctx.enter_context(tc.tile_pool(name="wf", bufs=2))
        outp = ctx.enter_context(tc.tile_pool(name="out", bufs=2))
        psS_p = ctx.enter_context(tc.tile_pool(name="psS", bufs=3, space="PSUM"))
        psO_p = ctx.enter_context(tc.tile_pool(name="psO", bufs=3, space="PSUM"))

        # ---- constants / precompute ----
        scales_sb = const.tile([G, NS], f32)
        nc.sync.dma_start(scales_sb[:], sc_d.ap())
        qz_sb = const.tile([G, NZS], i32)
        nc.sync.dma_start(qz_sb[:], qz_d.ap())

        # g_idx broadcast to 32 partitions
        g_b = const.tile([G, K], i32)
        nc.sync.dma_start(g_b[:], bass.AP(gi_d, 0, [[0, G], [1, K]]))

        iota_col = const.tile([G, 1], f32)
        nc.gpsimd.iota(iota_col[:], [[0, 1]], channel_multiplier=1,
                       allow_small_or_imprecise_dtypes=True)

        # one-hot in (t, j) block order: block u=t*8+j, col p <-> k = 1024t+8p+j
        # f32 then cast to bf16, replicated to partition strips 0/32/64.
        oh_f = const.tile([G, K], f32)
        g4 = g_b[:].rearrange("p (t q e) -> p t q e", t=KT, q=128, e=PF)
        for t in range(KT):
            for j in range(PF):
                u = t * PF + j
                nc.vector.tensor_scalar(
                    oh_f[:, u * 128:(u + 1) * 128], g4[:, t, :, j],
                    iota_col[:], None, op0=Alu.is_equal)
        onehot = const.tile([96, K], bf16)
        nc.vector.tensor_copy(onehot[0:G, :], oh_f[:])
        nc.sync.dma_start(onehot[32:64, :], onehot[0:32, :])
        nc.sync.dma_start(onehot[64:96, :], onehot[0:32, :])

        # unpack zeros (int32, strided by 8), then negsz = -(z * s)
        zeros_i = const.tile([G, NS], i32)
        z3 = zeros_i[:].rearrange("p (c e) -> p c e", e=PF)
        for jz in range(PF):
            nc.vector.tensor_scalar(
                z3[:, :, jz], qz_sb[:], 4 * jz, MAXQ,
                op0=Alu.logical_shift_right, op1=Alu.bitwise_and)
        negsz = const.tile([G, NS], f32)
        nc.vector.scalar_tensor_tensor(
            negsz[:], zeros_i[:], -1.0, scales_sb[:],
            op0=Alu.mult, op1=Alu.mult)

        # exact 3-way bf16 splits
        tmp_a = const.tile([G, NS], f32)
        tmp_b = const.tile([G, NS], f32)
        scombo = const.tile([96, NS], bf16)
        _split3(nc, scombo, scales_sb[:], tmp_a, tmp_b)
        nzcombo = const.tile([96, NS], bf16)
        _split3(nc, nzcombo, negsz[:], tmp_a, tmp_b)

        # ---- main loop ----
        for t in range(n_ktiles):
            qw_t = qwp.tile([128, NS], i32)
            nc.sync.dma_start(qw_t[:], qw_d.ap()[t * 128:(t + 1) * 128, :])
            for j in range(PF):
                u = t * PF + j
                wf = wfp.tile([128, NS], i32)
                nc.vector.tensor_scalar(
                    wf[:], qw_t[:], 4 * j, MAXQ,
                    op0=Alu.logical_shift_right, op1=Alu.bitwise_and)
                ot = outp.tile([128, NS], f32)
                oh_u = onehot[:, u * 128:(u + 1) * 128]
                for (c0, cw) in CHUNKS:
                    # single K=96 matmuls: 3 bf16 addend strips stacked along
                    # the contraction dim sum to the exact f32 row gather
                    psS = psS_p.tile([128, 512], f32, tag="psS")
                    nc.tensor.matmul(psS[:, :cw], oh_u,
                                     scombo[:, c0:c0 + cw],
                                     start=True, stop=True)
                    psZ = psO_p.tile([128, 512], f32, tag="psZ")
                    nc.tensor.matmul(psZ[:, :cw], oh_u,
                                     nzcombo[:, c0:c0 + cw],
                                     start=True, stop=True)
                    prod = wfp.tile([128, 512], f32, tag="prod")
                    nc.vector.tensor_tensor(
                        prod[:, :cw], wf[:, c0:c0 + cw], psS[:, :cw],
                        op=Alu.mult)
                    nc.vector.tensor_tensor(
                        ot[:, c0:c0 + cw], prod[:, :cw], psZ[:, :cw],
                        op=Alu.add)
                out4 = out_d.ap().rearrange("(t q e) n -> t q e n",
                                            t=KT, q=128, e=PF)
                nc.sync.dma_start(out4[t, :, j, :], ot[:])

    nc.compile()
    return nc


def get_module():
    if "nc" not in _module_cache:
        _module_cache["nc"] = build_module()
    return _module_cache["nc"]


def kernel(qweight, qzeros, scales, g_idx):
    qweight = np.ascontiguousarray(qweight, dtype=np.int32)
    qzeros = np.ascontiguousarray(qzeros, dtype=np.int32)
    scales = np.ascontiguousarray(scales, dtype=np.float32)
    g_idx_2d = np.ascontiguousarray(g_idx, dtype=np.int32).reshape(1, K)

    nc = get_module()
    in_maps = []
    for c in range(NCORES):
        nlo, nhi = c * NS, (c + 1) * NS
        in_maps.append({
            "qweight": np.ascontiguousarray(qweight[:, nlo:nhi]),
            "scales": np.ascontiguousarray(scales[:, nlo:nhi]),
            "qzeros": np.ascontiguousarray(qzeros[:, c * NZS:(c + 1) * NZS]),
            "g_idx": g_idx_2d,
        })
    res = run_bass_kernel_spmd(nc, in_maps, list(range(NCORES))).results
    out = np.concatenate([res[c]["out"] for c in range(NCORES)], axis=1)
    return np.ascontiguousarray(out, dtype=np.float32)


# revision 21
# speedup vs baseline: 1.9727x; 1.0868x over previous
"""GPTQ-style 4-bit dequantizer on 8 TRN2 NeuronCores.

Strategy (column-parallel per the N-axis sharding hint):
  - Shard qweight/scales/qzeros/output along N across 8 cores; g_idx replicated.
  - Per core (shard N_S = 1376 columns), natural row-major output layout:
      * unpack qzeros -> zeros; negsz = -(zeros*scales)  [32, N_S] (tiny)
      * split scales and negsz into 3 exact bf16 addends (8+8+8 mantissa bits)
      * one-hot(g_idx) [32, 4096] built on device, replicated to 3 partition
        strips so K=32 matmuls row-pack into the 128x128 PE array
      * for each 128-row packed tile t and shift j:
          w   = (qw_t >> 4j) & 0xF                    fused tensor_scalar (DVE)
          sS  = sum of 3 bf16 one-hot matmuls          -> PSUM (exact scales row gather)
          out = w * sS                                 one tensor_tensor (DVE) -> PSUM
          out += sum of 3 bf16 one-hot matmuls of negsz   (PE accumulates onto the
                DVE product; per-slot dummy matmuls set has_written once at start)
          ot  = copy(out)                              ACT PSUM->SBUF
      * strided-row DMA store (rows 8*kpf+j), 5504 B per row descriptor.
"""

import numpy as np
from contextlib import ExitStack

import concourse.bacc as bacc
import concourse.bass as bass
import concourse.tile as tile
import concourse.mybir as mybir
from concourse.bass_utils import run_bass_kernel_spmd

K = 4096          # input features (rows of dequantized weight)
N = 11008         # output features
G = 32            # quant groups
PF = 8            # int32 packs 8 nibbles
MAXQ = 0xF
NCORES = 8
NS = N // NCORES        # 1376 columns per core
KP = K // PF            # 512 packed rows
NZS = NS // PF          # 172 packed qzero columns per core
KT = KP // 128          # 4 packed row-tiles
CHUNKS = [(0, 512), (512, 512), (1024, NS - 1024)]  # matmul N<=512 chunks
NSLOT = 3               # persistent PSUM output slots

f32 = mybir.dt.float32
bf16 = mybir.dt.bfloat16
i32 = mybir.dt.int32
i16 = mybir.dt.int16
Alu = mybir.AluOpType

_module_cache = {}


def _split3(nc, dst, src, tmp_a, tmp_b):
    """dst[0:32]+dst[32:64]+dst[64:96] (bf16) == src (f32) exactly.
    Casts run on ACT to keep DVE free."""
    nc.scalar.copy(dst[0:32, :], src)                   # hi = bf16(src)
    nc.scalar.copy(tmp_a[:], dst[0:32, :])              # f32(hi)
    nc.vector.tensor_sub(tmp_b[:], src, tmp_a[:])       # r1 = src - hi (exact)
    nc.scalar.copy(dst[32:64, :], tmp_b[:])             # mid = bf16(r1)
    nc.scalar.copy(tmp_a[:], dst[32:64, :])             # f32(mid)
    nc.vector.tensor_sub(tmp_a[:], tmp_b[:], tmp_a[:])  # r2 = r1 - mid (exact)
    nc.scalar.copy(dst[64:96, :], tmp_a[:])             # lo = bf16(r2) == r2


def build_module(n_ktiles=KT):
    nc = bacc.Bacc("TRN2", target_bir_lowering=False, debug=False,
                   num_devices=NCORES)
    qw_d = nc.dram_tensor("qweight", [KP, NS], i32, kind="ExternalInput")
    sc_d = nc.dram_tensor("scales", [G, NS], f32, kind="ExternalInput")
    qz_d = nc.dram_tensor("qzeros", [G, NZS], i32, kind="ExternalInput")
    gi_d = nc.dram_tensor("g_idx", [1, K], i32, kind="ExternalInput")
    out_d = nc.dram_tensor("out", [K, NS], f32, kind="ExternalOutput")

    with tile.TileContext(nc) as tc, ExitStack() as ctx:
        const = ctx.enter_context(tc.tile_pool(name="const", bufs=1))
        qwp = ctx.enter_context(tc.tile_pool(name="qw", bufs=2))
        wfp = ctx.enter_context(tc.tile_pool(name="wf", bufs=2))
        outp = ctx.enter_context(tc.tile_pool(name="out", bufs=2))
        psS_p = ctx.enter_context(tc.tile_pool(name="psS", bufs=3, space="PSUM"))
        psO_p = ctx.enter_context(tc.tile_pool(name="psO", bufs=3, space="PSUM"))

        # ---- constants / precompute ----
        scales_sb = const.tile([G, NS], f32)
        nc.sync.dma_start(scales_sb[:], sc_d.ap())
        qz_sb = const.tile([G, NZS], i32)
        nc.sync.dma_start(qz_sb[:], qz_d.ap())

        # g_idx broadcast to 32 partitions
        g_b = const.tile([G, K], i32)
        nc.sync.dma_start(g_b[:], bass.AP(gi_d, 0, [[0, G], [1, K]]))

        iota_col = const.tile([G, 1], f32)
        nc.gpsimd.iota(iota_col[:], [[0, 1]], channel_multiplier=1,
                       allow_small_or_imprecise_dtypes=True)

        # one-hot in (t, j) block order: block u=t*8+j, col p <-> k = 1024t+8p+j
        # single tensor_scalar via a permuted view; cast on ACT; replicate to
        # partition strips 0/32/64 (K=96 stacked-contraction matmuls).
        oh_f = const.tile([G, K], f32)
        g5 = g_b[:].rearrange("p (t q e) -> p t e q", t=KT, q=128, e=PF)
        oh_v = oh_f[:].rearrange("p (t e q) -> p t e q", t=KT, e=PF, q=128)
        nc.vector.tensor_scalar(oh_v, g5, iota_col[:], None,
                                op0=Alu.is_equal)
        onehot = const.tile([96, K], bf16)
        nc.scalar.copy(onehot[0:G, :], oh_f[:])
        nc.sync.dma_start(onehot[32:64, :], onehot[0:32, :])
        nc.sync.dma_start(onehot[64:96, :], onehot[0:32, :])

        # unpack zeros (int32, strided by 8), then negsz = -(z * s)
        zeros_i = const.tile([G, NS], i32)
        z3 = zeros_i[:].rearrange("p (c e) -> p c e", e=PF)
        for jz in range(PF):
            nc.vector.tensor_scalar(
                z3[:, :, jz], qz_sb[:], 4 * jz, MAXQ,
                op0=Alu.logical_shift_right, op1=Alu.bitwise_and)
        negsz = const.tile([G, NS], f32)
        nc.vector.scalar_tensor_tensor(
            negsz[:], zeros_i[:], -1.0, scales_sb[:],
            op0=Alu.mult, op1=Alu.mult)

        # exact 3-way bf16 splits
        tmp_a = const.tile([G, NS], f32)
        tmp_b = const.tile([G, NS], f32)
        scombo = const.tile([96, NS], bf16)
        _split3(nc, scombo, scales_sb[:], tmp_a, tmp_b)
        nzcombo = const.tile([96, NS], bf16)
        _split3(nc, nzcombo, negsz[:], tmp_a, tmp_b)

        # ---- main loop ----
        # int16 view of the packed words: one 4x-mode tensor_scalar per shift
        # pair tt produces nibbles for j=tt (even halfwords) and j=tt+4 (odd).
        for t in range(n_ktiles):
            qw_t = qwp.tile([128, NS], i32)
            nc.sync.dma_start(qw_t[:], qw_d.ap()[t * 128:(t + 1) * 128, :])
            qw16 = qw_t[:].bitcast(i16)
            for tt in range(4):
                wf16 = wfp.tile([128, 2 * NS], i16, tag="wf16")
                nc.vector.tensor_scalar(
                    wf16[:], qw16, 4 * tt, MAXQ,
                    op0=Alu.logical_shift_right, op1=Alu.bitwise_and)
                wf3 = wf16[:].rearrange("p (c e) -> p c e", e=2)
                for l in range(2):
                    j = tt + 4 * l
                    u = t * PF + j
                    wfv = wf3[:, :, l]
                    ot = outp.tile([128, NS], f32)
                    oh_u = onehot[:, u * 128:(u + 1) * 128]
                    self_chunks(nc, psS_p, psO_p, wfp, oh_u, scombo, nzcombo,
                                wfv, ot)
                    out4 = out_d.ap().rearrange("(t q e) n -> t q e n",
                                                t=KT, q=128, e=PF)
                    nc.sync.dma_start(out4[t, :, j, :], ot[:])

    nc.compile()
    return nc


def self_chunks(nc, psS_p, psO_p, wfp, oh_u, scombo, nzcombo, wfv, ot):
    for (c0, cw) in CHUNKS:
        # single K=96 matmuls: 3 bf16 addend strips stacked along
        # the contraction dim sum to the exact f32 row gather
        psS = psS_p.tile([128, 512], f32, tag="psS")
        nc.tensor.matmul(psS[:, :cw], oh_u, scombo[:, c0:c0 + cw],
                         start=True, stop=True)
        psZ = psO_p.tile([128, 512], f32, tag="psZ")
        nc.tensor.matmul(psZ[:, :cw], oh_u, nzcombo[:, c0:c0 + cw],
                         start=True, stop=True)
        prod = wfp.tile([128, 512], f32, tag="prod")
        nc.vector.tensor_tensor(
            prod[:, :cw], wfv[:, c0:c0 + cw], psS[:, :cw], op=Alu.mult)
        nc.vector.tensor_tensor(
            ot[:, c0:c0 + cw], prod[:, :cw], psZ[:, :cw], op=Alu.add)


def get_module():
    if "nc" not in _module_cache:
        _module_cache["nc"] = build_module()
    return _module_cache["nc"]


def kernel(qweight, qzeros, scales, g_idx):
    qweight = np.ascontiguousarray(qweight, dtype=np.int32)
    qzeros = np.ascontiguousarray(qzeros, dtype=np.int32)
    scales = np.ascontiguousarray(scales, dtype=np.float32)
    g_idx_2d = np.ascontiguousarray(g_idx, dtype=np.int32).reshape(1, K)

    nc = get_module()
    in_maps = []
    for c in range(NCORES):
        nlo, nhi = c * NS, (c + 1) * NS
        in_maps.append({
            "qweight": np.ascontiguousarray(qweight[:, nlo:nhi]),
            "scales": np.ascontiguousarray(scales[:, nlo:nhi]),
            "qzeros": np.ascontiguousarray(qzeros[:, c * NZS:(c + 1) * NZS]),
            "g_idx": g_idx_2d,
        })
    res = run_bass_kernel_spmd(nc, in_maps, list(range(NCORES))).results
    out = np.concatenate([res[c]["out"] for c in range(NCORES)], axis=1)
    return np.ascontiguousarray(out, dtype=np.float32)


# revision 24
# speedup vs baseline: 2.0868x; 1.0578x over previous
"""GPTQ-style 4-bit dequantizer on 8 TRN2 NeuronCores.

Strategy (column-parallel per the N-axis sharding hint):
  - Shard qweight/scales/qzeros/output along N across 8 cores; g_idx replicated.
  - Per core (shard N_S = 1376 columns), natural row-major output layout:
      * unpack qzeros -> zeros; negsz = -(zeros*scales)  [32, N_S] (tiny)
      * split scales and negsz into 3 exact bf16 addends (8+8+8 mantissa bits)
      * one-hot(g_idx) [32, 4096] built on device, replicated to 3 partition
        strips so K=32 matmuls row-pack into the 128x128 PE array
      * for each 128-row packed tile t and shift j:
          w   = (qw_t >> 4j) & 0xF                    fused tensor_scalar (DVE)
          sS  = sum of 3 bf16 one-hot matmuls          -> PSUM (exact scales row gather)
          out = w * sS                                 one tensor_tensor (DVE) -> PSUM
          out += sum of 3 bf16 one-hot matmuls of negsz   (PE accumulates onto the
                DVE product; per-slot dummy matmuls set has_written once at start)
          ot  = copy(out)                              ACT PSUM->SBUF
      * strided-row DMA store (rows 8*kpf+j), 5504 B per row descriptor.
"""

import numpy as np
from contextlib import ExitStack

import concourse.bacc as bacc
import concourse.bass as bass
import concourse.tile as tile
import concourse.mybir as mybir
from concourse.bass_utils import run_bass_kernel_spmd

K = 4096          # input features (rows of dequantized weight)
N = 11008         # output features
G = 32            # quant groups
PF = 8            # int32 packs 8 nibbles
MAXQ = 0xF
NCORES = 8
NS = N // NCORES        # 1376 columns per core
KP = K // PF            # 512 packed rows
NZS = NS // PF          # 172 packed qzero columns per core
KT = KP // 128          # 4 packed row-tiles
CHUNKS = [(0, 688), (688, 688)]        # per-chunk: 2-bank PSUM tiles
MMSPLIT = [(0, 512), (512, 176)]       # matmul N<=512 sub-tiles per chunk

f32 = mybir.dt.float32
bf16 = mybir.dt.bfloat16
i32 = mybir.dt.int32
i16 = mybir.dt.int16
Alu = mybir.AluOpType

_module_cache = {}


def _split3(nc, dst, src, tmp_a, tmp_b):
    """dst[0:32]+dst[32:64]+dst[64:96] (bf16) == src (f32) exactly.
    Casts run on ACT to keep DVE free."""
    nc.scalar.copy(dst[0:32, :], src)                   # hi = bf16(src)
    nc.scalar.copy(tmp_a[:], dst[0:32, :])              # f32(hi)
    nc.vector.tensor_sub(tmp_b[:], src, tmp_a[:])       # r1 = src - hi (exact)
    nc.scalar.copy(dst[32:64, :], tmp_b[:])             # mid = bf16(r1)
    nc.scalar.copy(tmp_a[:], dst[32:64, :])             # f32(mid)
    nc.vector.tensor_sub(tmp_a[:], tmp_b[:], tmp_a[:])  # r2 = r1 - mid (exact)
    nc.scalar.copy(dst[64:96, :], tmp_a[:])             # lo = bf16(r2) == r2


def build_module(n_ktiles=KT):
    nc = bacc.Bacc("TRN2", target_bir_lowering=False, debug=False,
                   num_devices=NCORES)
    qw_d = nc.dram_tensor("qweight", [KP, NS], i32, kind="ExternalInput")
    sc_d = nc.dram_tensor("scales", [G, NS], f32, kind="ExternalInput")
    qz_d = nc.dram_tensor("qzeros", [G, NZS], i32, kind="ExternalInput")
    gi_d = nc.dram_tensor("g_idx", [1, K], i32, kind="ExternalInput")
    out_d = nc.dram_tensor("out", [K, NS], f32, kind="ExternalOutput")

    with tile.TileContext(nc) as tc, ExitStack() as ctx:
        const = ctx.enter_context(tc.tile_pool(name="const", bufs=1))
        qwp = ctx.enter_context(tc.tile_pool(name="qw", bufs=2))
        wfp = ctx.enter_context(tc.tile_pool(name="wf", bufs=2))
        outp = ctx.enter_context(tc.tile_pool(name="out", bufs=2))
        psS_p = ctx.enter_context(tc.tile_pool(name="psS", bufs=2, space="PSUM"))
        psO_p = ctx.enter_context(tc.tile_pool(name="psO", bufs=2, space="PSUM"))

        # ---- constants / precompute ----
        scales_sb = const.tile([G, NS], f32)
        nc.sync.dma_start(scales_sb[:], sc_d.ap())
        qz_sb = const.tile([G, NZS], i32)
        nc.sync.dma_start(qz_sb[:], qz_d.ap())

        # g_idx broadcast to 32 partitions
        g_b = const.tile([G, K], i32)
        nc.sync.dma_start(g_b[:], bass.AP(gi_d, 0, [[0, G], [1, K]]))

        iota_col = const.tile([G, 1], f32)
        nc.gpsimd.iota(iota_col[:], [[0, 1]], channel_multiplier=1,
                       allow_small_or_imprecise_dtypes=True)

        # one-hot in (t, j) block order: block u=t*8+j, col p <-> k = 1024t+8p+j
        # single tensor_scalar via a permuted view; cast on ACT; replicate to
        # partition strips 0/32/64 (K=96 stacked-contraction matmuls).
        oh_f = const.tile([G, K], f32)
        g5 = g_b[:].rearrange("p (t q e) -> p t e q", t=KT, q=128, e=PF)
        oh_v = oh_f[:].rearrange("p (t e q) -> p t e q", t=KT, e=PF, q=128)
        nc.vector.tensor_scalar(oh_v, g5, iota_col[:], None,
                                op0=Alu.is_equal)
        onehot = const.tile([96, K], bf16)
        nc.scalar.copy(onehot[0:G, :], oh_f[:])
        nc.sync.dma_start(onehot[32:64, :], onehot[0:32, :])
        nc.sync.dma_start(onehot[64:96, :], onehot[0:32, :])

        # unpack zeros (int32, strided by 8), then negsz = -(z * s)
        zeros_i = const.tile([G, NS], i32)
        z3 = zeros_i[:].rearrange("p (c e) -> p c e", e=PF)
        for jz in range(PF):
            nc.vector.tensor_scalar(
                z3[:, :, jz], qz_sb[:], 4 * jz, MAXQ,
                op0=Alu.logical_shift_right, op1=Alu.bitwise_and)
        negsz = const.tile([G, NS], f32)
        nc.vector.scalar_tensor_tensor(
            negsz[:], zeros_i[:], -1.0, scales_sb[:],
            op0=Alu.mult, op1=Alu.mult)

        # exact 3-way bf16 splits
        tmp_a = const.tile([G, NS], f32)
        tmp_b = const.tile([G, NS], f32)
        scombo = const.tile([96, NS], bf16)
        _split3(nc, scombo, scales_sb[:], tmp_a, tmp_b)
        nzcombo = const.tile([96, NS], bf16)
        _split3(nc, nzcombo, negsz[:], tmp_a, tmp_b)

        # ---- main loop ----
        # int16 view of the packed words: one 4x-mode tensor_scalar per shift
        # pair tt produces nibbles for j=tt (even halfwords) and j=tt+4 (odd).
        for t in range(n_ktiles):
            qw_t = qwp.tile([128, NS], i32)
            nc.sync.dma_start(qw_t[:], qw_d.ap()[t * 128:(t + 1) * 128, :])
            qw16 = qw_t[:].bitcast(i16)
            for tt in range(4):
                wf16 = wfp.tile([128, 2 * NS], i16, tag="wf16")
                nc.vector.tensor_scalar(
                    wf16[:], qw16, 4 * tt, MAXQ,
                    op0=Alu.logical_shift_right, op1=Alu.bitwise_and)
                wf3 = wf16[:].rearrange("p (c e) -> p c e", e=2)
                for l in range(2):
                    j = tt + 4 * l
                    u = t * PF + j
                    wfv = wf3[:, :, l]
                    ot = outp.tile([128, NS], f32)
                    oh_u = onehot[:, u * 128:(u + 1) * 128]
                    self_chunks(nc, psS_p, psO_p, wfp, oh_u, scombo, nzcombo,
                                wfv, ot)
                    out4 = out_d.ap().rearrange("(t q e) n -> t q e n",
                                                t=KT, q=128, e=PF)
                    nc.sync.dma_start(out4[t, :, j, :], ot[:])

    nc.compile()
    return nc


def self_chunks(nc, psS_p, psO_p, wfp, oh_u, scombo, nzcombo, wfv, ot):
    for (c0, cw) in CHUNKS:
        # single K=96 matmuls: 3 bf16 addend strips stacked along
        # the contraction dim sum to the exact f32 row gather
        psS = psS_p.tile([128, 688], f32, tag="psS")
        psZ = psO_p.tile([128, 688], f32, tag="psZ")
        for (m0, mw) in MMSPLIT:
            nc.tensor.matmul(psS[:, m0:m0 + mw], oh_u,
                             scombo[:, c0 + m0:c0 + m0 + mw],
                             start=True, stop=True)
            nc.tensor.matmul(psZ[:, m0:m0 + mw], oh_u,
                             nzcombo[:, c0 + m0:c0 + m0 + mw],
                             start=True, stop=True)
        prod = wfp.tile([128, 688], f32, tag="prod")
        nc.vector.tensor_tensor(
            prod[:, :cw], wfv[:, c0:c0 + cw], psS[:, :cw], op=Alu.mult)
        nc.vector.tensor_tensor(
            ot[:, c0:c0 + cw], prod[:, :cw], psZ[:, :cw], op=Alu.add)


def get_module():
    if "nc" not in _module_cache:
        _module_cache["nc"] = build_module()
    return _module_cache["nc"]


def kernel(qweight, qzeros, scales, g_idx):
    qweight = np.ascontiguousarray(qweight, dtype=np.int32)
    qzeros = np.ascontiguousarray(qzeros, dtype=np.int32)
    scales = np.ascontiguousarray(scales, dtype=np.float32)
    g_idx_2d = np.ascontiguousarray(g_idx, dtype=np.int32).reshape(1, K)

    nc = get_module()
    in_maps = []
    for c in range(NCORES):
        nlo, nhi = c * NS, (c + 1) * NS
        in_maps.append({
            "qweight": np.ascontiguousarray(qweight[:, nlo:nhi]),
            "scales": np.ascontiguousarray(scales[:, nlo:nhi]),
            "qzeros": np.ascontiguousarray(qzeros[:, c * NZS:(c + 1) * NZS]),
            "g_idx": g_idx_2d,
        })
    res = run_bass_kernel_spmd(nc, in_maps, list(range(NCORES))).results
    out = np.concatenate([res[c]["out"] for c in range(NCORES)], axis=1)
    return np.ascontiguousarray(out, dtype=np.float32)


# revision 26
# speedup vs baseline: 2.0972x; 1.0050x over previous
"""GPTQ-style 4-bit dequantizer on 8 TRN2 NeuronCores.

Strategy (column-parallel per the N-axis sharding hint):
  - Shard qweight/scales/qzeros/output along N across 8 cores; g_idx replicated.
  - Per core (shard N_S = 1376 columns), natural row-major output layout:
      * unpack qzeros -> zeros; negsz = -(zeros*scales)  [32, N_S] (tiny)
      * split scales and negsz into 3 exact bf16 addends (8+8+8 mantissa bits)
      * one-hot(g_idx) [32, 4096] built on device, replicated to 3 partition
        strips so K=32 matmuls row-pack into the 128x128 PE array
      * for each 128-row packed tile t and shift j:
          w   = (qw_t >> 4j) & 0xF                    fused tensor_scalar (DVE)
          sS  = sum of 3 bf16 one-hot matmuls          -> PSUM (exact scales row gather)
          out = w * sS                                 one tensor_tensor (DVE) -> PSUM
          out += sum of 3 bf16 one-hot matmuls of negsz   (PE accumulates onto the
                DVE product; per-slot dummy matmuls set has_written once at start)
          ot  = copy(out)                              ACT PSUM->SBUF
      * strided-row DMA store (rows 8*kpf+j), 5504 B per row descriptor.
"""

import numpy as np
from contextlib import ExitStack

import concourse.bacc as bacc
import concourse.bass as bass
import concourse.tile as tile
import concourse.mybir as mybir
from concourse.bass_utils import run_bass_kernel_spmd

K = 4096          # input features (rows of dequantized weight)
N = 11008         # output features
G = 32            # quant groups
PF = 8            # int32 packs 8 nibbles
MAXQ = 0xF
NCORES = 8
NS = N // NCORES        # 1376 columns per core
KP = K // PF            # 512 packed rows
NZS = NS // PF          # 172 packed qzero columns per core
KT = KP // 128          # 4 packed row-tiles
CHUNKS = [(0, 688), (688, 688)]        # per-chunk: 2-bank PSUM tiles
MMSPLIT = [(0, 512), (512, 176)]       # matmul N<=512 sub-tiles per chunk

f32 = mybir.dt.float32
bf16 = mybir.dt.bfloat16
i32 = mybir.dt.int32
i16 = mybir.dt.int16
Alu = mybir.AluOpType

_module_cache = {}


def _split3(nc, dst, src, tmp_a, tmp_b):
    """dst[0:32]+dst[32:64]+dst[64:96] (bf16) == src (f32) exactly.
    Casts run on ACT to keep DVE free."""
    nc.scalar.copy(dst[0:32, :], src)                   # hi = bf16(src)
    nc.scalar.copy(tmp_a[:], dst[0:32, :])              # f32(hi)
    nc.vector.tensor_sub(tmp_b[:], src, tmp_a[:])       # r1 = src - hi (exact)
    nc.scalar.copy(dst[32:64, :], tmp_b[:])             # mid = bf16(r1)
    nc.scalar.copy(tmp_a[:], dst[32:64, :])             # f32(mid)
    nc.vector.tensor_sub(tmp_a[:], tmp_b[:], tmp_a[:])  # r2 = r1 - mid (exact)
    nc.scalar.copy(dst[64:96, :], tmp_a[:])             # lo = bf16(r2) == r2


def build_module(n_ktiles=KT):
    nc = bacc.Bacc("TRN2", target_bir_lowering=False, debug=False,
                   num_devices=NCORES)
    qw_d = nc.dram_tensor("qweight", [KP, NS], i32, kind="ExternalInput")
    sc_d = nc.dram_tensor("scales", [G, NS], f32, kind="ExternalInput")
    qz_d = nc.dram_tensor("qzeros", [G, NZS], i32, kind="ExternalInput")
    gi_d = nc.dram_tensor("g_idx", [1, K], i32, kind="ExternalInput")
    out_d = nc.dram_tensor("out", [K, NS], f32, kind="ExternalOutput")

    with tile.TileContext(nc) as tc, ExitStack() as ctx:
        const = ctx.enter_context(tc.tile_pool(name="const", bufs=1))
        qwp = ctx.enter_context(tc.tile_pool(name="qw", bufs=2))
        wfp = ctx.enter_context(tc.tile_pool(name="wf", bufs=3))
        outp = ctx.enter_context(tc.tile_pool(name="out", bufs=3))
        psS_p = ctx.enter_context(tc.tile_pool(name="psS", bufs=2, space="PSUM"))
        psO_p = ctx.enter_context(tc.tile_pool(name="psO", bufs=2, space="PSUM"))

        # ---- constants / precompute ----
        scales_sb = const.tile([G, NS], f32)
        nc.sync.dma_start(scales_sb[:], sc_d.ap())
        qz_sb = const.tile([G, NZS], i32)
        nc.sync.dma_start(qz_sb[:], qz_d.ap())

        # g_idx broadcast to 32 partitions
        g_b = const.tile([G, K], i32)
        nc.sync.dma_start(g_b[:], bass.AP(gi_d, 0, [[0, G], [1, K]]))

        iota_col = const.tile([G, 1], f32)
        nc.gpsimd.iota(iota_col[:], [[0, 1]], channel_multiplier=1,
                       allow_small_or_imprecise_dtypes=True)

        # one-hot in (t, j) block order: block u=t*8+j, col p <-> k = 1024t+8p+j
        # single tensor_scalar via a permuted view; cast on ACT; replicate to
        # partition strips 0/32/64 (K=96 stacked-contraction matmuls).
        oh_f = const.tile([G, K], f32)
        g5 = g_b[:].rearrange("p (t q e) -> p t e q", t=KT, q=128, e=PF)
        oh_v = oh_f[:].rearrange("p (t e q) -> p t e q", t=KT, e=PF, q=128)
        nc.vector.tensor_scalar(oh_v, g5, iota_col[:], None,
                                op0=Alu.is_equal)
        onehot = const.tile([96, K], bf16)
        nc.scalar.copy(onehot[0:G, :], oh_f[:])
        nc.sync.dma_start(onehot[32:64, :], onehot[0:32, :])
        nc.sync.dma_start(onehot[64:96, :], onehot[0:32, :])

        # unpack zeros (int32, strided by 8), then negsz = -(z * s)
        zeros_i = const.tile([G, NS], i32)
        z3 = zeros_i[:].rearrange("p (c e) -> p c e", e=PF)
        for jz in range(PF):
            nc.vector.tensor_scalar(
                z3[:, :, jz], qz_sb[:], 4 * jz, MAXQ,
                op0=Alu.logical_shift_right, op1=Alu.bitwise_and)
        negsz = const.tile([G, NS], f32)
        nc.vector.scalar_tensor_tensor(
            negsz[:], zeros_i[:], -1.0, scales_sb[:],
            op0=Alu.mult, op1=Alu.mult)

        # exact 3-way bf16 splits
        tmp_a = const.tile([G, NS], f32)
        tmp_b = const.tile([G, NS], f32)
        scombo = const.tile([96, NS], bf16)
        _split3(nc, scombo, scales_sb[:], tmp_a, tmp_b)
        nzcombo = const.tile([96, NS], bf16)
        _split3(nc, nzcombo, negsz[:], tmp_a, tmp_b)

        # PE warm-up: ~4.5us of back-to-back matmuls so HAM reaches 8/8
        # (2.4 GHz) before the gather matmuls start; overlaps input DMAs.
        warm = psS_p.tile([128, 688], f32, tag="psS")
        for _ in range(20):
            nc.tensor.matmul(warm[:, 0:512], onehot[:, 0:128],
                             scombo[:, 0:512], start=True, stop=True)

        # ---- main loop ----
        # int16 view of the packed words: one 4x-mode tensor_scalar per shift
        # pair tt produces nibbles for j=tt (even halfwords) and j=tt+4 (odd).
        for t in range(n_ktiles):
            qw_t = qwp.tile([128, NS], i32)
            nc.sync.dma_start(qw_t[:], qw_d.ap()[t * 128:(t + 1) * 128, :])
            qw16 = qw_t[:].bitcast(i16)
            for tt in range(4):
                wf16 = wfp.tile([128, 2 * NS], i16, tag="wf16")
                nc.vector.tensor_scalar(
                    wf16[:], qw16, 4 * tt, MAXQ,
                    op0=Alu.logical_shift_right, op1=Alu.bitwise_and)
                wf3 = wf16[:].rearrange("p (c e) -> p c e", e=2)
                for l in range(2):
                    j = tt + 4 * l
                    u = t * PF + j
                    wfv = wf3[:, :, l]
                    ot = outp.tile([128, NS], f32)
                    oh_u = onehot[:, u * 128:(u + 1) * 128]
                    self_chunks(nc, psS_p, psO_p, wfp, oh_u, scombo, nzcombo,
                                wfv, ot)
                    out4 = out_d.ap().rearrange("(t q e) n -> t q e n",
                                                t=KT, q=128, e=PF)
                    nc.sync.dma_start(out4[t, :, j, :], ot[:])

    nc.compile()
    return nc


def self_chunks(nc, psS_p, psO_p, wfp, oh_u, scombo, nzcombo, wfv, ot):
    for (c0, cw) in CHUNKS:
        # single K=96 matmuls: 3 bf16 addend strips stacked along
        # the contraction dim sum to the exact f32 row gather
        psS = psS_p.tile([128, 688], f32, tag="psS")
        psZ = psO_p.tile([128, 688], f32, tag="psZ")
        for (m0, mw) in MMSPLIT:
            nc.tensor.matmul(psS[:, m0:m0 + mw], oh_u,
                             scombo[:, c0 + m0:c0 + m0 + mw],
                             start=True, stop=True)
            nc.tensor.matmul(psZ[:, m0:m0 + mw], oh_u,
                             nzcombo[:, c0 + m0:c0 + m0 + mw],
                             start=True, stop=True)
        prod = wfp.tile([128, 688], f32, tag="prod")
        nc.vector.tensor_tensor(
            prod[:, :cw], wfv[:, c0:c0 + cw], psS[:, :cw], op=Alu.mult)
        nc.vector.tensor_tensor(
            ot[:, c0:c0 + cw], prod[:, :cw], psZ[:, :cw], op=Alu.add)


def get_module():
    if "nc" not in _module_cache:
        _module_cache["nc"] = build_module()
    return _module_cache["nc"]


def kernel(qweight, qzeros, scales, g_idx):
    qweight = np.ascontiguousarray(qweight, dtype=np.int32)
    qzeros = np.ascontiguousarray(qzeros, dtype=np.int32)
    scales = np.ascontiguousarray(scales, dtype=np.float32)
    g_idx_2d = np.ascontiguousarray(g_idx, dtype=np.int32).reshape(1, K)

    nc = get_module()
    in_maps = []
    for c in range(NCORES):
        nlo, nhi = c * NS, (c + 1) * NS
        in_maps.append({
            "qweight": np.ascontiguousarray(qweight[:, nlo:nhi]),
            "scales": np.ascontiguousarray(scales[:, nlo:nhi]),
            "qzeros": np.ascontiguousarray(qzeros[:, c * NZS:(c + 1) * NZS]),
            "g_idx": g_idx_2d,
        })
    res = run_bass_kernel_spmd(nc, in_maps, list(range(NCORES))).results
    out = np.concatenate([res[c]["out"] for c in range(NCORES)], axis=1)
    return np.ascontiguousarray(out, dtype=np.float32)
